# revision 34
# baseline (speedup 1.0000x reference)
"""Trainium2 Bass kernel for nn_EdgeEncoder (moe_routing).

Strategy (v3)
-------------
Each of E edges is routed to 1 of 9 expert MLPs (4 -> 256 -> 256), then
  out = relu(concat([type_embed[tid], source_embed[sid], pv]) @ Wf + bf).

Host (numpy, cheap O(E) work):
  * scale/mask params, group edge indices by expert (base type), split every
    expert's edges evenly over the 8 cores (per-expert segments need NO
    alignment: only the tail block is padded), so all cores run ONE program,
  * algebraic fusions so the device does minimal work:
      - V[t] = W2[t] @ Wf_pv (f64 host precompute) fuses layer 2 with the
        final projection: pv @ Wf_pv == h @ V[t] + const,
      - G = [type_embed @ Wf_t + (b2@Wf_pv + bf)[base] ; source_embed @ Wf_s]
        folds both embedding gathers and every bias into ONE shared 19-row
        one-hot matmul (const row folded into the type rows, so G no longer
        depends on the expert -> no per-expert instruction splits),
      - layer 1 is likewise shared: x is scattered into a 45-row
        block-diagonal layout (expert t owns rows 5t..5t+4, last row = 1.0
        carrying b1), against one [45, 256] stationary for all experts.
  * x-rows and u-rows are packed into ONE [64, L] bf16 device input.

Device per 512-edge block (edges pre-grouped by expert), all bf16 matmuls
with K padded to 128 (uniform PE tile_size: switching the stationary tile
size costs ~100-170 ns per switch on HW):
  hT   = relu(WG_l1.T @ in)             2 matmuls N=512    (PSUM 2 banks)
  outT = WG_g.T @ in (+) V[t].T @ hT    2 + 4 matmuls      (PSUM 2 banks)
  relu PSUM->SBUF bf16 on DVE, one [128,2,512] store per block; the host
  casts bf16 -> f32 and un-permutes.

Issue order is software-pipelined (L1/G of block b+1 issue before V of
block b) so the PE never waits on the h-activation; PSUM uses all 8 banks;
input DMAs are enqueued ahead of the bulk V-table DMA so the first block
starts ~4 us earlier (DMA engines drain rings roughly FIFO).
"""

import math
import os

import ml_dtypes
import numpy as np

import concourse.bacc as bacc
import concourse.bass as bass
import concourse.mybir as mybir
import concourse.tile as tile
from concourse.bass_utils import run_bass_kernel_spmd

# ---- static module configuration (mirrors the torch source) ----
T = 9            # base types ("experts")
P_MAX = 4
D = 256
N_TYPES = 14
N_SRC = 5
NCORES = 8
BLOCK = 512      # edges per device block (one PSUM bank of fp32 per n-half)
GRP = 128

BASE_MAP = np.array([0, 0, 0, 1, 1, 1, 2, 2, 3, 4, 5, 6, 7, 8], dtype=np.int32)
PCOUNT = np.array([2, 2, 1, 1, 1, 1, 3, 2, 4], dtype=np.int32)
SCALES = np.ones((T, P_MAX), dtype=np.float32)
SCALES[0, :2] = [1.0, 1e-06]      # nmos  m, w
SCALES[1, :2] = [1.0, 1e-06]      # pmos  m, w
SCALES[2, 0] = 1.0                # balun rout
SCALES[3, 0] = 1000.0             # resistor r
SCALES[4, 0] = 1e-12              # capacitor c
SCALES[5, 0] = 1e-09              # inductor l
SCALES[6, :3] = [1.0, 1.0, 1.0]   # vsource dc, mag, phase
SCALES[7, :2] = [0.001, 0.001]    # isource dc, mag
SCALES[8, :4] = [1.0, 1.0, 1e9, 1.0]  # port dbm, dc, freq, num

KX = T * (P_MAX + 1)              # 45 x-rows: block-diagonal per expert
KU = N_TYPES + N_SRC              # 19 one-hot rows (const folded into type)
KIN = KX + KU                     # 64 rows DMA'd; tile K-padded to 128

_F32 = mybir.dt.float32
_BF16 = mybir.dt.bfloat16
_NP_BF16 = ml_dtypes.bfloat16

NIB = 6                           # input-tile ring (2 iterations of WAR slack)
PREF = 4                          # input DMA prefetch depth (blocks)
NWARM = int(os.environ.get("EDGEENC_WARM", "9"))

_PROGRAM_CACHE: dict = {}
LAST_RESULT = None  # BassKernelResults of the most recent run (for harness)


def _layout(base_ids: np.ndarray):
    """Per-expert per-core segment sizes (no alignment, no tail padding: the
    final block is simply partial), identical on every core so one program
    serves all 8."""
    n_t = np.bincount(base_ids, minlength=T)
    m_t = np.array([math.ceil(int(n) / NCORES) for n in n_t], dtype=np.int64)
    L = int(m_t.sum())
    return n_t, m_t, L


def _build_order(base_ids: np.ndarray, n_t, m_t, L) -> np.ndarray:
    """ORD[c, j] = global edge index at per-core slot j (or -1 = pad)."""
    ORD = np.full((NCORES, L), -1, dtype=np.int64)
    off = 0
    for t in range(T):
        if m_t[t] == 0:
            continue
        seg = int(m_t[t])
        idx = np.nonzero(base_ids == t)[0]
        arr = np.full(NCORES * seg, -1, dtype=np.int64)
        arr[: idx.shape[0]] = idx
        ORD[:, off : off + seg] = arr.reshape(NCORES, seg)
        off += seg
    return ORD


def _host_inputs(type_ids, source_ids, params, ORD):
    """INP[c] = [64, L] bf16: 45 block-diagonal x-rows + 19 one-hot u-rows."""
    base_ids = BASE_MAP[type_ids]
    scales = SCALES[base_ids]                                  # [E,4]
    validp = np.arange(P_MAX)[None, :] < PCOUNT[base_ids][:, None]
    x = np.where(validp, params.astype(np.float32) / scales, 0.0)

    L = ORD.shape[1]
    # full 128 K-rows (upper 64 stay zero) so no device-side pad memsets
    # are needed: the DMA writes whole tiles.
    INP = np.zeros((NCORES, L, GRP), dtype=np.float32)
    valid = ORD >= 0
    ids = ORD[valid]
    ci, co = np.nonzero(valid)
    be = base_ids[ids]
    INP[ci[:, None], co[:, None], (5 * be)[:, None] + np.arange(P_MAX)[None, :]] = x[ids]
    INP[ci, co, 5 * be + P_MAX] = 1.0                          # b1 / ones row
    INP[ci, co, KX + type_ids[ids]] = 1.0
    INP[ci, co, KX + N_TYPES + source_ids[ids]] = 1.0
    return np.ascontiguousarray(INP.transpose(0, 2, 1)).astype(_NP_BF16)


def _host_weights(type_embed, source_embed, W1, b1, W2, b2, Wf, bf):
    f = np.float32
    W1 = np.asarray(W1, f); b1 = np.asarray(b1, f)
    W2 = np.asarray(W2, np.float64)
    b2 = np.asarray(b2, f); Wf = np.asarray(Wf, f); bf = np.asarray(bf, f)
    type_embed = np.asarray(type_embed, f); source_embed = np.asarray(source_embed, f)

    Wft, Wfs, Wfp = Wf[:D], Wf[D : 2 * D], Wf[2 * D :]

    # layer-1 stationary: [45, 256], expert t rows 5t..5t+4 (b1 in last row)
    W1e45 = np.concatenate([W1, b1[:, None, :]], axis=1).reshape(KX, D)

    # shared G [19, 256]: type rows carry the per-expert const row
    gc = b2 @ Wfp + bf[None, :]                                # [9,256]
    gt = type_embed @ Wft + gc[BASE_MAP]                       # [14,256]
    gs = source_embed @ Wfs                                    # [5,256]
    G19 = np.concatenate([gt, gs], axis=0)                     # [19,256]

    # one [128, 512] bf16 stationary: cols 0:256 layer-1, 256:512 G
    # (K padded to 128 with zeros so the PE tile_size never changes)
    WG = np.zeros((GRP, 2 * D), dtype=f)
    WG[0:KX, 0:D] = W1e45
    WG[KX:KIN, D : 2 * D] = G19
    WG = WG.astype(_NP_BF16)

    # V[t] = W2[t] @ Wf_pv (f64), fusing layer 2 with the final projection.
    # bf16 chunks [128, T*2*2*128]: [:, ((t*2+k)*2+g)*128+m] = V[t][k*128+p, g*128+m]
    V = (W2 @ Wfp.astype(np.float64)).astype(f)                # [9,256,256]
    VR = np.ascontiguousarray(
        V.reshape(T, 2, GRP, 2, GRP).transpose(2, 0, 1, 3, 4).reshape(GRP, T * 4 * GRP)
    ).astype(_NP_BF16)
    return WG, VR


def _block_runs(m_t: np.ndarray, L: int, NB: int):
    """Per block: list of (expert, col0, col1) runs of equal expert, with
    arbitrary (unaligned) run boundaries; the final block is partial."""
    bounds = np.cumsum(np.asarray(m_t))                        # segment ends
    out = []
    for b in range(NB):
        lo, hi = b * BLOCK, min((b + 1) * BLOCK, L)
        runs = []
        for t in range(T):
            s0 = 0 if t == 0 else int(bounds[t - 1])
            s1 = int(bounds[t])
            c0, c1 = max(lo, s0), min(hi, s1)
            if c0 < c1:
                runs.append((t, c0 - lo, c1 - lo))
        out.append(runs)
    return out


def _build_program(m_t: tuple, L: int):
    key = (m_t, L)
    if key in _PROGRAM_CACHE:
        return _PROGRAM_CACHE[key]

    NB = math.ceil(L / BLOCK)
    runs_per_block = _block_runs(np.asarray(m_t, dtype=np.int64), L, NB)
    width = lambda b: min(BLOCK, L - b * BLOCK)

    nc = bacc.Bacc("TRN2", target_bir_lowering=False, debug=False,
                   num_devices=NCORES)
    inp_d = nc.dram_tensor("inp", [GRP, L], _BF16, kind="ExternalInput")
    wg_d = nc.dram_tensor("wg", [GRP, 2 * D], _BF16, kind="ExternalInput")
    vr_d = nc.dram_tensor("vr", [GRP, T * 4 * GRP], _BF16, kind="ExternalInput")
    out_d = nc.dram_tensor("out", [GRP, 2, L], _BF16, kind="ExternalOutput")

    RELU = mybir.ActivationFunctionType.Relu

    with tile.TileContext(nc) as tc:
        with (
            tc.tile_pool(name="wts", bufs=1) as wts,
            tc.tile_pool(name="inp", bufs=1) as inp,
            tc.tile_pool(name="hcb", bufs=1) as hcb,
            tc.tile_pool(name="osb", bufs=1) as osbp,
            tc.tile_pool(name="hps", bufs=1, space=bass.MemorySpace.PSUM) as hps,
            tc.tile_pool(name="ops", bufs=1, space=bass.MemorySpace.PSUM) as ops,
        ):
            # K is padded to 128 everywhere (rows 64:128 zero, zeros come in
            # with the DMA) so the PE never switches stationary tile_size.
            wgt = wts.tile([GRP, 2 * D], _BF16)
            vrt = wts.tile([GRP, T * 4 * GRP], _BF16)
            wrm = wts.tile([GRP, BLOCK], _BF16, name="wrm")

            # the Relu act-table load is auto-inserted at the head of the ACT
            # queue (no waits), so it overlaps the initial DMAs for free.
            # The warm-up scratch memset runs on GpSimd as its first
            # instruction so the PE clock ramp starts ~0.3us earlier; the
            # DMA gens it delays have ample slack.
            if NWARM:
                nc.gpsimd.memset(wrm[:], 0.0)

            ints = [inp.tile([GRP, BLOCK], _BF16, name=f"int{j}")
                    for j in range(NIB)]
            hcats = [hcb.tile([GRP, 2, BLOCK], _BF16, name=f"hcat{j}")
                     for j in range(4)]
            osbs = [osbp.tile([GRP, 2, BLOCK], _BF16, name=f"osb{j}")
                    for j in range(4)]
            htss = [hps.tile([GRP, 2, BLOCK], _F32, name=f"hts{j}")
                    for j in range(2)]
            otss = [ops.tile([GRP, 2, BLOCK], _F32, name=f"ots{j}")
                    for j in range(2)]

            # processing order: the tiny tail block (if any) is processed
            # FIRST — in the tail position its near-zero PE work collapses
            # the pipeline's WAR slack (observed 1.2us stall), while at the
            # head it rides the clock-ramp phase for free. The final drain
            # then belongs to a full block whose relu/store can be split.
            seq = list(range(NB))
            if NB >= 2 and width(NB - 1) < 256:
                seq = [NB - 1] + seq[: NB - 1]

            def dma_in(p, eng=None):
                blk = seq[p]
                W = width(blk)
                (eng or nc.gpsimd).dma_start(
                    ints[p % NIB][:, 0:W],
                    inp_d.ap()[:, blk * BLOCK : blk * BLOCK + W])

            # load order matters: the DMA engines drain rings roughly FIFO,
            # so the first input tiles must be enqueued ahead of the V table,
            # and V goes as per-expert 128KB chunks (expert 0 first) so no
            # single bulk transfer head-of-line blocks a soon-needed input
            # tile for more than ~0.4us. int0 + V ride the sync queue, whose
            # descriptor generation is otherwise idle until the first store.
            dma_in(0, nc.sync)
            nc.gpsimd.dma_start(wgt[:], wg_d.ap())
            for b in range(1, min(PREF, NB)):
                dma_in(b)
            # V chunks in first-needed expert order given the block sequence
            need = []
            for p in range(NB):
                for (t, _, _) in runs_per_block[seq[p]]:
                    if t not in need:
                        need.append(t)
            for t in need:
                c0, c1 = t * 4 * GRP, (t + 1) * 4 * GRP
                nc.sync.dma_start(vrt[:, c0:c1], vr_d.ap()[:, c0:c1])

            # a few garbage matmuls ramp the PE p-state while input DMAs land
            for i in range(NWARM):
                nc.tensor.matmul(htss[0][:, i % 2, :], wrm[:, 0:GRP], wrm[:],
                                 start=True, stop=True)

            def issue_L1(p):
                if p + PREF < NB:
                    dma_in(p + PREF)
                blk = seq[p]
                W = width(blk)
                hts = htss[p % 2]
                it = ints[p % NIB]
                for g in (0, 1):
                    nc.tensor.matmul(
                        hts[:, g, 0:W] if W < BLOCK else hts[:, g, :],
                        wgt[:, g * GRP : (g + 1) * GRP],
                        it[:, 0:W] if W < BLOCK else it[:],
                        start=True, stop=True)
                if W < BLOCK:
                    nc.scalar.activation(hcats[p % 4][:, :, 0:W],
                                         hts[:, :, 0:W], RELU)
                else:
                    nc.scalar.activation(hcats[p % 4][:], hts[:], RELU)

            def issue_G(p):
                blk = seq[p]
                W = width(blk)
                ots = otss[p % 2]
                it = ints[p % NIB]
                for g in (0, 1):
                    nc.tensor.matmul(
                        ots[:, g, 0:W] if W < BLOCK else ots[:, g, :],
                        wgt[:, D + g * GRP : D + (g + 1) * GRP],
                        it[:, 0:W] if W < BLOCK else it[:],
                        start=True, stop=False, skip_group_check=True)

            def issue_V(p):
                blk = seq[p]
                W = width(blk)
                ots = otss[p % 2]
                hc = hcats[p % 4]
                osb = osbs[p % 4]
                runs = runs_per_block[blk]
                for ri, (t, c0, c1) in enumerate(runs):
                    last = ri == len(runs) - 1
                    for g in (0, 1):
                        for k in (0, 1):
                            nc.tensor.matmul(
                                ots[:, g, c0:c1],
                                vrt[:, ((t * 2 + k) * 2 + g) * GRP
                                    : ((t * 2 + k) * 2 + g + 1) * GRP],
                                hc[:, k, c0:c1],
                                start=False, stop=last and k == 1,
                                skip_group_check=True)
                if p == NB - 1 and W >= 256:
                    # final drain: split in halves so the first store's gen
                    # and wire overlap the second relu. Keep everything on
                    # DVE+sync — extra cross-engine hops at the drain cost
                    # more in semaphore latency than they save in overlap.
                    for g in (0, 1):
                        nc.vector.tensor_scalar_max(
                            osb[:, g, 0:W], ots[:, g, 0:W], 0.0)
                        nc.sync.dma_start(
                            out_d.ap()[:, g, blk * BLOCK : blk * BLOCK + W],
                            osb[:, g, 0:W])
                elif W < BLOCK:
                    nc.vector.tensor_scalar_max(
                        osb[:, :, 0:W], ots[:, :, 0:W], 0.0)
                    nc.sync.dma_start(
                        out_d.ap()[:, :, blk * BLOCK : blk * BLOCK + W],
                        osb[:, :, 0:W])
                else:
                    nc.vector.tensor_scalar_max(osb[:], ots[:], 0.0)
                    nc.sync.dma_start(
                        out_d.ap()[:, :, blk * BLOCK : (blk + 1) * BLOCK],
                        osb[:])

            # per-cycle PE order [L1(i), V(i-2), G(i-1)]: a full extra cycle
            # separates the DVE relu of block b from the G matmul that
            # recycles its PSUM tile, so that WAR never stalls the PE.
            for i in range(NB + 2):
                if i < NB:
                    issue_L1(i)
                if i >= 2:
                    issue_V(i - 2)
                if 1 <= i <= NB:
                    issue_G(i - 1)

    nc.compile()
    _PROGRAM_CACHE[key] = nc
    return nc


def kernel(type_ids, source_ids, params, type_embed, source_embed,
           W1, b1, W2, b2, Wf, bf):
    global LAST_RESULT
    type_ids = np.asarray(type_ids, dtype=np.int32)
    source_ids = np.asarray(source_ids, dtype=np.int32)
    params = np.asarray(params, dtype=np.float32)
    E = type_ids.shape[0]

    base_ids = BASE_MAP[type_ids]
    n_t, m_t, L = _layout(base_ids)
    ORD = _build_order(base_ids, n_t, m_t, L)
    INP = _host_inputs(type_ids, source_ids, params, ORD)
    WG, VR = _host_weights(type_embed, source_embed, W1, b1, W2, b2, Wf, bf)

    nc = _build_program(tuple(int(v) for v in m_t), L)

    in_maps = [{"inp": np.ascontiguousarray(INP[c]), "wg": WG, "vr": VR}
               for c in range(NCORES)]

    trace = bool(int(os.environ.get("EDGEENC_TRACE", "0")))
    res = run_bass_kernel_spmd(nc, in_maps, core_ids=list(range(NCORES)),
                               trace=trace)
    LAST_RESULT = res

    full = np.zeros((E, D), dtype=np.float32)
    for c in range(NCORES):
        sel = ORD[c] >= 0
        oc = np.asarray(res.results[c]["out"]).astype(np.float32)  # [128,2,L]
        oc = oc.transpose(1, 0, 2).reshape(D, L)
        full[ORD[c][sel]] = np.ascontiguousarray(oc[:, sel].T)
    return full
